# revision 14
# baseline (speedup 1.0000x reference)
"""RBM local-operator kernel for Trainium2 (8 NeuronCores, SPMD).

Math: for y_k = x with spin k flipped (x in {-1,+1}^N),
  logpsi(y_k) - logpsi(x) = -2 x_k a_k + S1_k + sum_h log(1 - x_k t_h tau_kh)
with th = xW + b, t = tanh(th), tau = tanh(2W), S1_k = sum_h logcosh(2W_kh).
Since |2W| <= ~0.1, tau = 2W to 3e-6 absolute, and |t*tau| <= ~0.09 so
log(1-u) = -(u + u^2/2 + ...) truncated at n=2 is accurate to ~1e-4 in the
exponent.  Per-core (H sliced 8 ways, 256 each):
  qo = sum_h t * 2W        (M1)        qe = sum_h (t^2/2) * (2W)^2   (M2)
Host combines: out = exp(S1 - sum_c qe - x * (sum_c qo + 2a)) @ Oxy, with
S1 computed on host (it depends only on W).  All matmul/vector work is
bf16; output fp16.  End-to-end rel err ~2.5e-3 vs f64 (gate 2e-2).

Raw bass (no tile framework), tuned for DMA latency: the theta input
(both W h-tiles + x chunks + biases, one packed tensor) is split in two
chunk-half DMAs over the sync and scalar rings so the theta gate lands
~1us earlier than a single transfer; g1a rides the gpsimd SWDGE ring
(needed first by the M1 matmuls), g1b rides second on scalar (needed
last).  Output DMAs are fire-and-forget - they complete during the fixed
~7us semaphore-restore postamble the NEFF wrapper appends instead of
serializing before it.  The PE spins zero-dep warm-up matmuls (garbage
data into a dummy PSUM bank) from instruction 0 to ramp the HAM clock
gate.  All q accumulators are half-width so the PSUM->SBUF copies split
evenly across the scalar and vector engines.
"""

import sys

import numpy as np

_BASS_REPO = "/opt/trn_rl_repo"
if _BASS_REPO not in sys.path:
    sys.path.insert(0, _BASS_REPO)

from contextlib import ExitStack

from concourse import bacc, mybir
from concourse.bass_utils import run_bass_kernel_spmd

B, N, H, NCORES = 64, 512, 2048, 8
HL = H // NCORES          # hidden slice per core: 256
HT = HL // 128            # SBUF partition tiles per slice: 2
CCH = N // 128            # theta contraction chunks: 4
CW = 256 + B + HT         # wax chunk: 128 Wa + 128 Wb + 64 x + 2 bias
F32 = mybir.dt.float32
F16 = mybir.dt.float16
BF16 = mybir.dt.bfloat16
AF = mybir.ActivationFunctionType

N_WARM_PRE = 6
N_WARM_POST = 1
RSQRT2 = 0.7071067811865476

_CACHE = {}


def _build_bass():
    nc = bacc.Bacc(
        "TRN2", target_bir_lowering=False, debug=False, num_devices=NCORES
    )
    # wax[p, c, 0:128] = W chunk c h-tile a; [128:256] = h-tile b;
    # [256:320] = x chunk c; [320:322] = biases (same in every chunk).
    wax_d = nc.declare_dram_parameter("wax", [128, CCH, CW], BF16, isOutput=False)
    g1a_d = nc.declare_dram_parameter("g1a", [128, N], BF16, isOutput=False)
    g1b_d = nc.declare_dram_parameter("g1b", [128, N], BF16, isOutput=False)
    q_d = nc.declare_dram_parameter("q", [B, 2, N], F16, isOutput=True)

    with ExitStack() as ctx:
        e = ctx.enter_context
        wax = e(nc.sbuf_tensor([128, CCH, CW], BF16))
        g1a = e(nc.sbuf_tensor([128, N], BF16))
        g1b = e(nc.sbuf_tensor([128, N], BF16))
        zz = e(nc.sbuf_tensor([128, N], BF16))   # never written: garbage is fine
        T1a = e(nc.sbuf_tensor([128, B], BF16))
        T1b = e(nc.sbuf_tensor([128, B], BF16))
        T2a = e(nc.sbuf_tensor([128, B], BF16))
        T2b = e(nc.sbuf_tensor([128, B], BF16))
        G2a = e(nc.sbuf_tensor([128, N], BF16))
        G2b = e(nc.sbuf_tensor([128, N], BF16))
        qo_sb = e(nc.sbuf_tensor([B, N], F16))
        qe_sb = e(nc.sbuf_tensor([B, N], F16))

        NH = N // 2
        dummy = e(nc.psum_tensor([B, N], F32))
        qoL = e(nc.psum_tensor([B, NH], F32))
        qoR = e(nc.psum_tensor([B, NH], F32))
        qeL = e(nc.psum_tensor([B, NH], F32))
        qeR = e(nc.psum_tensor([B, NH], F32))
        tha = e(nc.psum_tensor([128, B], F32))
        thb = e(nc.psum_tensor([128, B], F32))

        dWa = e(nc.semaphore())   # wax chunks 0-1 (+ biases)
        dWc = e(nc.semaphore())   # wax chunks 2-3
        dGa = e(nc.semaphore())
        dGb = e(nc.semaphore())
        dmaOut = e(nc.semaphore())
        pe = e(nc.semaphore())
        act = e(nc.semaphore())
        dve = e(nc.semaphore())

        # ---- SYNC ring: wax chunks 0-1 in; qo out ---------------------
        nc.sync.dma_start(wax[:, 0:2, :], wax_d[:, 0:2, :]).then_inc(dWa, 16)
        nc.sync.wait_ge(dve, 4)   # qoL copied
        nc.sync.wait_ge(act, 4)   # qoR copied
        nc.sync.dma_start(q_d[:, 1, :], qo_sb[:]).then_inc(dmaOut, 16)

        # ---- GPSIMD (SWDGE ring): g1a in ------------------------------
        nc.gpsimd.dma_start(g1a[:], g1a_d[:]).then_inc(dGa, 16)

        # ---- SCALAR ring: wax chunks 2-3, g1b in; qe out --------------
        nc.scalar.dma_start(wax[:, 2:4, :], wax_d[:, 2:4, :]).then_inc(dWc, 16)
        nc.scalar.dma_start(g1b[:], g1b_d[:]).then_inc(dGb, 16)
        nc.scalar.wait_ge(pe, 1)
        nc.scalar.wait_ge(dWa, 16)
        nc.scalar.activation(
            T1a[:], tha[:], AF.Tanh, bias=wax[:, 0, 320:321]
        ).then_inc(act, 1)
        nc.scalar.wait_ge(pe, 2)
        nc.scalar.activation(
            T1b[:], thb[:], AF.Tanh, bias=wax[:, 0, 321:322]
        ).then_inc(act, 1)
        # T2b = (t_b/sqrt(2))^2 on the same engine: no cross-engine hop
        nc.scalar.activation(T2b[:], T1b[:], AF.Square, scale=RSQRT2).then_inc(
            act, 1
        )
        nc.scalar.wait_ge(pe, 4)
        nc.scalar.copy(qo_sb[:, NH:N], qoR[:]).then_inc(act, 1)
        nc.scalar.wait_ge(pe, 6)
        nc.scalar.copy(qe_sb[:, NH:N], qeR[:]).then_inc(act, 1)
        nc.scalar.wait_ge(dve, 5)  # qeL copied
        nc.scalar.dma_start(q_d[:, 0, :], qe_sb[:]).then_inc(dmaOut, 16)

        # ---- VECTOR: G2a, T2a, G2b, qoL/qeL copies --------------------
        nc.vector.wait_ge(dGa, 16)
        nc.vector.tensor_mul(G2a[:], g1a[:], g1a[:]).then_inc(dve, 1)
        nc.vector.wait_ge(act, 1)
        nc.vector.scalar_tensor_tensor(
            T2a[:], T1a[:], 0.5, T1a[:], mybir.AluOpType.mult, mybir.AluOpType.mult
        ).then_inc(dve, 1)
        nc.vector.wait_ge(dGb, 16)
        nc.vector.tensor_mul(G2b[:], g1b[:], g1b[:]).then_inc(dve, 1)
        nc.vector.wait_ge(pe, 3)
        nc.vector.tensor_copy(qo_sb[:, 0:NH], qoL[:]).then_inc(dve, 1)
        nc.vector.wait_ge(pe, 5)
        nc.vector.tensor_copy(qe_sb[:, 0:NH], qeL[:]).then_inc(dve, 1)

        # ---- PE ------------------------------------------------------
        # Warm-up spins on garbage from instruction 0 (no deps).
        for i in range(N_WARM_PRE):
            nc.tensor.matmul(
                dummy[:], zz[:, :B], zz[:], start=(i == 0), stop=False
            )
        # thetaT[h, b] = sum_n W[n, h] x[n, b]   (h on partitions)
        nc.tensor.wait_ge(dWa, 16)
        for t, thp in enumerate((tha, thb)):
            for c in range(CCH):
                if t == 0 and c == 2:
                    nc.tensor.wait_ge(dWc, 16)
                mm = nc.tensor.matmul(
                    thp[:], wax[:, c, t * 128 : (t + 1) * 128],
                    wax[:, c, 256 : 256 + B],
                    start=(c == 0), stop=(c == CCH - 1),
                )
            mm.then_inc(pe, 1)
        for i in range(N_WARM_POST):
            nc.tensor.matmul(dummy[:], zz[:, :B], zz[:], start=False, stop=True)
        # M1 into qoL/qoR, M2 into qeL/qeR (half-width banks)
        nc.tensor.wait_ge(act, 1)
        nc.tensor.wait_ge(dGa, 16)
        nc.tensor.matmul(qoL[:], T1a[:], g1a[:, 0:NH], start=True, stop=False)
        nc.tensor.matmul(qoR[:], T1a[:], g1a[:, NH:N], start=True, stop=False)
        nc.tensor.wait_ge(dve, 2)
        nc.tensor.matmul(qeL[:], T2a[:], G2a[:, 0:NH], start=True, stop=False)
        nc.tensor.matmul(qeR[:], T2a[:], G2a[:, NH:N], start=True, stop=False)
        nc.tensor.wait_ge(act, 2)
        nc.tensor.wait_ge(dGb, 16)
        nc.tensor.matmul(
            qoL[:], T1b[:], g1b[:, 0:NH], start=False, stop=True
        ).then_inc(pe, 1)
        nc.tensor.matmul(
            qoR[:], T1b[:], g1b[:, NH:N], start=False, stop=True
        ).then_inc(pe, 1)
        nc.tensor.wait_ge(act, 3)
        nc.tensor.wait_ge(dve, 3)
        nc.tensor.matmul(
            qeL[:], T2b[:], G2b[:, 0:NH], start=False, stop=True
        ).then_inc(pe, 1)
        nc.tensor.matmul(
            qeR[:], T2b[:], G2b[:, NH:N], start=False, stop=True
        ).then_inc(pe, 1)

    nc.compile()
    return nc


def _get_bass():
    if "nc" not in _CACHE:
        _CACHE["nc"] = _build_bass()
    return _CACHE["nc"]


def _prep_inputs(x, W, b, a):
    """Per-core input maps. All host-side layout prep."""
    import ml_dtypes

    bf16 = ml_dtypes.bfloat16
    x = np.asarray(x, dtype=np.float32)
    W = np.asarray(W, dtype=np.float32)
    b = np.asarray(b, dtype=np.float32)

    xtb = np.ascontiguousarray(
        x.T.reshape(CCH, 128, B).transpose(1, 0, 2)
    )  # [128, CCH, B]; xtb[p, c, bb] = x[bb, c*128 + p]

    in_maps = []
    for c in range(NCORES):
        sl = slice(c * HL, (c + 1) * HL)
        Wc = W[:, sl]  # [N, HL]
        # wc[p, t, ch, h] = W[ch*128+p, c*HL + t*128 + h]
        wc = Wc.reshape(CCH, 128, HT, 128).transpose(1, 2, 0, 3)
        wax = np.empty((128, CCH, CW), dtype=bf16)
        wax[:, :, 0:128] = wc[:, 0].astype(bf16)
        wax[:, :, 128:256] = wc[:, 1].astype(bf16)
        wax[:, :, 256 : 256 + B] = xtb.astype(bf16)
        bt = b[sl].reshape(HT, 128).T.astype(bf16)  # [128, HT]
        wax[:, :, 256 + B : CW] = bt[:, None, :]
        g1t = (2.0 * Wc).T.reshape(HT, 128, N).transpose(1, 0, 2)
        g1a = np.ascontiguousarray(g1t[:, 0]).astype(bf16)
        g1b = np.ascontiguousarray(g1t[:, 1]).astype(bf16)
        in_maps.append({"wax": wax, "g1a": g1a, "g1b": g1b})
    return in_maps


def _combine(x, W, a, Oxy, results):
    x = np.asarray(x, dtype=np.float64)
    W = np.asarray(W, dtype=np.float64)
    a = np.asarray(a, dtype=np.float64)
    Oxy = np.asarray(Oxy, dtype=np.float64)
    q = np.zeros((B, 2, N), dtype=np.float64)
    for r in results:
        q += r["q"].astype(np.float64)
    z = 2.0 * W
    az = np.abs(z)
    S1 = (az + np.log1p(np.exp(-2.0 * az)) - np.log(2.0)).sum(axis=1)  # [N]
    E = np.exp(S1[None, :] - q[:, 0, :] - x * (q[:, 1, :] + 2.0 * a[None, :]))
    return (E @ Oxy).astype(np.float32)


def kernel(x, W, b, a, Oxy):
    nc = _get_bass()
    in_maps = _prep_inputs(x, W, b, a)
    res = run_bass_kernel_spmd(nc, in_maps, list(range(NCORES))).results
    return _combine(x, W, a, Oxy, res)


# revision 15
# speedup vs baseline: 1.0937x; 1.0937x over previous
"""RBM local-operator kernel for Trainium2 (8 NeuronCores, SPMD).

Math: for y_k = x with spin k flipped (x in {-1,+1}^N),
  logpsi(y_k) - logpsi(x) = -2 x_k a_k + S1_k + sum_h log(1 - x_k t_h tau_kh)
with th = xW + b, t = tanh(th), tau = tanh(2W), S1_k = sum_h logcosh(2W_kh).
Since |2W| <= ~0.1, tau = 2W to 3e-6 absolute, and |t*tau| <= ~0.09 so
log(1-u) = -(u + u^2/2 + ...) truncated at n=2 is accurate to ~1e-4 in the
exponent.  Per-core (H sliced 8 ways, 256 each):
  qo = sum_h t * 2W        (M1)        qe = sum_h (t^2/2) * (2W)^2   (M2)
Host combines: out = exp(S1 - sum_c qe - x * (sum_c qo + 2a)) @ Oxy, with
S1 computed on host (it depends only on W).  All matmul/vector work is
bf16; output fp16.  End-to-end rel err ~2.5e-3 vs f64 (gate 2e-2).

Raw bass (no tile framework), tuned for DMA latency: the theta input
(both W h-tiles + x chunks + biases, one packed tensor) is split in two
chunk-half DMAs over the sync and scalar rings so the theta gate lands
~1us earlier than a single transfer; g1a rides the gpsimd SWDGE ring
(needed first by the M1 matmuls), g1b rides second on scalar (needed
last).  Output DMAs are fire-and-forget - they complete during the fixed
~7us semaphore-restore postamble the NEFF wrapper appends instead of
serializing before it.  The PE spins zero-dep warm-up matmuls (garbage
data into a dummy PSUM bank) from instruction 0 to ramp the HAM clock
gate.  All q accumulators are half-width so the PSUM->SBUF copies split
evenly across the scalar and vector engines.
"""

import sys

import numpy as np

_BASS_REPO = "/opt/trn_rl_repo"
if _BASS_REPO not in sys.path:
    sys.path.insert(0, _BASS_REPO)

from contextlib import ExitStack

from concourse import bacc, mybir
from concourse.bass_utils import run_bass_kernel_spmd

B, N, H, NCORES = 64, 512, 2048, 8
HL = H // NCORES          # hidden slice per core: 256
HT = HL // 128            # SBUF partition tiles per slice: 2
CCH = N // 128            # theta contraction chunks: 4
CW = 256 + B + HT         # wax chunk: 128 Wa + 128 Wb + 64 x + 2 bias
F32 = mybir.dt.float32
F16 = mybir.dt.float16
BF16 = mybir.dt.bfloat16
AF = mybir.ActivationFunctionType

N_WARM_PRE = 8
N_WARM_POST = 1
RSQRT2 = 0.7071067811865476

_CACHE = {}


def _build_bass():
    nc = bacc.Bacc(
        "TRN2", target_bir_lowering=False, debug=False, num_devices=NCORES
    )
    # wax[p, c, 0:128] = W chunk c h-tile a; [128:256] = h-tile b;
    # [256:320] = x chunk c; [320:322] = biases (same in every chunk).
    wax_d = nc.declare_dram_parameter("wax", [128, CCH, CW], BF16, isOutput=False)
    g1a_d = nc.declare_dram_parameter("g1a", [128, N], BF16, isOutput=False)
    g1b_d = nc.declare_dram_parameter("g1b", [128, N], BF16, isOutput=False)
    q_d = nc.declare_dram_parameter("q", [B, 2, N], F16, isOutput=True)

    with ExitStack() as ctx:
        e = ctx.enter_context
        wax = e(nc.sbuf_tensor([128, CCH, CW], BF16))
        g1a = e(nc.sbuf_tensor([128, N], BF16))
        g1b = e(nc.sbuf_tensor([128, N], BF16))
        zz = e(nc.sbuf_tensor([128, N], BF16))   # never written: garbage is fine
        T1a = e(nc.sbuf_tensor([128, B], BF16))
        T1b = e(nc.sbuf_tensor([128, B], BF16))
        T2a = e(nc.sbuf_tensor([128, B], BF16))
        T2b = e(nc.sbuf_tensor([128, B], BF16))
        G2a = e(nc.sbuf_tensor([128, N], BF16))
        G2b = e(nc.sbuf_tensor([128, N], BF16))
        q_sb = e(nc.sbuf_tensor([B, 2, N], F16))

        NH = N // 2
        dummy = e(nc.psum_tensor([B, N], F32))
        qoL = e(nc.psum_tensor([B, NH], F32))
        qoR = e(nc.psum_tensor([B, NH], F32))
        qeL = e(nc.psum_tensor([B, NH], F32))
        qeR = e(nc.psum_tensor([B, NH], F32))
        tha = e(nc.psum_tensor([128, B], F32))
        thb = e(nc.psum_tensor([128, B], F32))

        dWa = e(nc.semaphore())   # wax chunks 0-1 (+ biases)
        dWc = e(nc.semaphore())   # wax chunks 2-3
        dGa = e(nc.semaphore())
        dGb = e(nc.semaphore())
        dmaOut = e(nc.semaphore())
        pe = e(nc.semaphore())
        act = e(nc.semaphore())
        dve = e(nc.semaphore())

        # ---- SYNC ring: wax chunks 0-1 in; qo out ---------------------
        nc.sync.dma_start(wax[:, 0:2, :], wax_d[:, 0:2, :]).then_inc(dWa, 16)
        nc.sync.wait_ge(dve, 5)   # qoL + qeL copied
        nc.sync.wait_ge(act, 5)   # qoR + qeR copied
        nc.sync.dma_start(q_d[:], q_sb[:]).then_inc(dmaOut, 16)

        # ---- GPSIMD (SWDGE ring): g1a in ------------------------------
        nc.gpsimd.dma_start(g1a[:], g1a_d[:]).then_inc(dGa, 16)

        # ---- SCALAR ring: wax chunks 2-3, g1b in; qe out --------------
        nc.scalar.dma_start(wax[:, 2:4, :], wax_d[:, 2:4, :]).then_inc(dWc, 16)
        nc.scalar.dma_start(g1b[:], g1b_d[:]).then_inc(dGb, 16)
        nc.scalar.wait_ge(pe, 1)
        nc.scalar.wait_ge(dWa, 16)
        nc.scalar.activation(
            T1a[:], tha[:], AF.Tanh, bias=wax[:, 0, 320:321]
        ).then_inc(act, 1)
        nc.scalar.wait_ge(pe, 2)
        nc.scalar.activation(
            T1b[:], thb[:], AF.Tanh, bias=wax[:, 0, 321:322]
        ).then_inc(act, 1)
        # T2b = (t_b/sqrt(2))^2 on the same engine: no cross-engine hop
        nc.scalar.activation(T2b[:], T1b[:], AF.Square, scale=RSQRT2).then_inc(
            act, 1
        )
        nc.scalar.wait_ge(pe, 4)
        nc.scalar.copy(q_sb[:, 1, NH:N], qoR[:]).then_inc(act, 1)
        nc.scalar.wait_ge(pe, 6)
        nc.scalar.copy(q_sb[:, 0, NH:N], qeR[:]).then_inc(act, 1)

        # ---- VECTOR: G2a, T2a, G2b, qoL/qeL copies --------------------
        nc.vector.wait_ge(dGa, 16)
        nc.vector.tensor_mul(G2a[:], g1a[:], g1a[:]).then_inc(dve, 1)
        nc.vector.wait_ge(act, 1)
        nc.vector.scalar_tensor_tensor(
            T2a[:], T1a[:], 0.5, T1a[:], mybir.AluOpType.mult, mybir.AluOpType.mult
        ).then_inc(dve, 1)
        nc.vector.wait_ge(dGb, 16)
        nc.vector.tensor_mul(G2b[:], g1b[:], g1b[:]).then_inc(dve, 1)
        nc.vector.wait_ge(pe, 3)
        nc.vector.tensor_copy(q_sb[:, 1, 0:NH], qoL[:]).then_inc(dve, 1)
        nc.vector.wait_ge(pe, 5)
        nc.vector.tensor_copy(q_sb[:, 0, 0:NH], qeL[:]).then_inc(dve, 1)

        # ---- PE ------------------------------------------------------
        # Warm-up spins on garbage from instruction 0 (no deps).
        for i in range(N_WARM_PRE):
            nc.tensor.matmul(
                dummy[:], zz[:, :B], zz[:], start=(i == 0), stop=False
            )
        # thetaT[h, b] = sum_n W[n, h] x[n, b]   (h on partitions)
        nc.tensor.wait_ge(dWa, 16)
        for t, thp in enumerate((tha, thb)):
            for c in range(CCH):
                if t == 0 and c == 2:
                    nc.tensor.wait_ge(dWc, 16)
                mm = nc.tensor.matmul(
                    thp[:], wax[:, c, t * 128 : (t + 1) * 128],
                    wax[:, c, 256 : 256 + B],
                    start=(c == 0), stop=(c == CCH - 1),
                )
            mm.then_inc(pe, 1)
        for i in range(N_WARM_POST):
            nc.tensor.matmul(dummy[:], zz[:, :B], zz[:], start=False, stop=True)
        # M1 into qoL/qoR, M2 into qeL/qeR (half-width banks)
        nc.tensor.wait_ge(act, 1)
        nc.tensor.wait_ge(dGa, 16)
        nc.tensor.matmul(qoL[:], T1a[:], g1a[:, 0:NH], start=True, stop=False)
        nc.tensor.matmul(qoR[:], T1a[:], g1a[:, NH:N], start=True, stop=False)
        nc.tensor.wait_ge(dve, 2)
        nc.tensor.matmul(qeL[:], T2a[:], G2a[:, 0:NH], start=True, stop=False)
        nc.tensor.matmul(qeR[:], T2a[:], G2a[:, NH:N], start=True, stop=False)
        nc.tensor.wait_ge(act, 2)
        nc.tensor.wait_ge(dGb, 16)
        nc.tensor.matmul(
            qoL[:], T1b[:], g1b[:, 0:NH], start=False, stop=True
        ).then_inc(pe, 1)
        nc.tensor.matmul(
            qoR[:], T1b[:], g1b[:, NH:N], start=False, stop=True
        ).then_inc(pe, 1)
        nc.tensor.wait_ge(act, 3)
        nc.tensor.wait_ge(dve, 3)
        nc.tensor.matmul(
            qeL[:], T2b[:], G2b[:, 0:NH], start=False, stop=True
        ).then_inc(pe, 1)
        nc.tensor.matmul(
            qeR[:], T2b[:], G2b[:, NH:N], start=False, stop=True
        ).then_inc(pe, 1)

    nc.compile()
    return nc


def _get_bass():
    if "nc" not in _CACHE:
        _CACHE["nc"] = _build_bass()
    return _CACHE["nc"]


def _prep_inputs(x, W, b, a):
    """Per-core input maps. All host-side layout prep."""
    import ml_dtypes

    bf16 = ml_dtypes.bfloat16
    x = np.asarray(x, dtype=np.float32)
    W = np.asarray(W, dtype=np.float32)
    b = np.asarray(b, dtype=np.float32)

    xtb = np.ascontiguousarray(
        x.T.reshape(CCH, 128, B).transpose(1, 0, 2)
    )  # [128, CCH, B]; xtb[p, c, bb] = x[bb, c*128 + p]

    in_maps = []
    for c in range(NCORES):
        sl = slice(c * HL, (c + 1) * HL)
        Wc = W[:, sl]  # [N, HL]
        # wc[p, t, ch, h] = W[ch*128+p, c*HL + t*128 + h]
        wc = Wc.reshape(CCH, 128, HT, 128).transpose(1, 2, 0, 3)
        wax = np.empty((128, CCH, CW), dtype=bf16)
        wax[:, :, 0:128] = wc[:, 0].astype(bf16)
        wax[:, :, 128:256] = wc[:, 1].astype(bf16)
        wax[:, :, 256 : 256 + B] = xtb.astype(bf16)
        bt = b[sl].reshape(HT, 128).T.astype(bf16)  # [128, HT]
        wax[:, :, 256 + B : CW] = bt[:, None, :]
        g1t = (2.0 * Wc).T.reshape(HT, 128, N).transpose(1, 0, 2)
        g1a = np.ascontiguousarray(g1t[:, 0]).astype(bf16)
        g1b = np.ascontiguousarray(g1t[:, 1]).astype(bf16)
        in_maps.append({"wax": wax, "g1a": g1a, "g1b": g1b})
    return in_maps


def _combine(x, W, a, Oxy, results):
    x = np.asarray(x, dtype=np.float64)
    W = np.asarray(W, dtype=np.float64)
    a = np.asarray(a, dtype=np.float64)
    Oxy = np.asarray(Oxy, dtype=np.float64)
    q = np.zeros((B, 2, N), dtype=np.float64)
    for r in results:
        q += r["q"].astype(np.float64)
    z = 2.0 * W
    az = np.abs(z)
    S1 = (az + np.log1p(np.exp(-2.0 * az)) - np.log(2.0)).sum(axis=1)  # [N]
    E = np.exp(S1[None, :] - q[:, 0, :] - x * (q[:, 1, :] + 2.0 * a[None, :]))
    return (E @ Oxy).astype(np.float32)


def kernel(x, W, b, a, Oxy):
    nc = _get_bass()
    in_maps = _prep_inputs(x, W, b, a)
    res = run_bass_kernel_spmd(nc, in_maps, list(range(NCORES))).results
    return _combine(x, W, a, Oxy, res)
